# revision 10
# baseline (speedup 1.0000x reference)
"""Trainium2 Bass kernel for nn_DynamicFc (per-sample dynamic MLP).

Data-parallel over 8 cores (batch 8192 -> 8 x 1024), 8 tiles of 128/core.

Per 128-sample tile (natural [batch, feat] layout):
  psum_fl = f @ [Wf.T | Wf.T@B1] + [bf | bf@B1]        (PE, 160 cols)
  pf_lowT = (Wpf.T).T-chunks @ pfT-chunks + bpf        (PE, transposed [j,b])
  p1 = pf_low @ W1 (l-major cols), p2 = pf_low @ W2 (m-major)   (PE)
  prod1 = p1 * f_low   (DVE/GPSIMD, routed per chunk; scalar copies
                        psum->sbuf bf16 so DVE can run the 2x mode via a
                        pair-duplicated broadcast operand)
  h = relu(tree_reduce_l(prod1) + h_bias)              (DVE tree + scalar)
  prod2 = p2 * h; g = tree_reduce_m(prod2)             (same scheme)
  out = gT.T @ fin1 + [hT;1].T @ fin2 + r16            (PE + DVE adds)
Residual r16 = (f+pf) in bf16 from host; output stored bf16, upcast on host.
"""

import os
import sys

import numpy as np

for _p in ("/opt/trn_rl_repo", "/root/.axon_site/_ro/trn_rl_repo"):
    if os.path.isdir(_p) and _p not in sys.path:
        sys.path.insert(0, _p)

import bass_rust
import concourse.bass as bass
import concourse.bass_utils as _bu
import concourse.mybir as mybir
import concourse.tile as tile
from concourse.bass_utils import run_bass_kernel_spmd
from concourse.masks import make_identity

import ml_dtypes

N_CORES = 8
B, D, LOW, MID = 8192, 1024, 128, 32
DIM = LOW * MID  # 4096
SHARD = B // N_CORES  # 1024
TILE_B = 128
NT = SHARD // TILE_B  # 8
NCHUNK = D // 128  # 8

F32 = mybir.dt.float32
CDT = mybir.dt.bfloat16
NP_CDT = ml_dtypes.bfloat16

_CACHED = {}

# per-chunk routing for the params first-touch multiply (4 chunks of 1024
# per half): "V" = DVE direct from psum (1x), "SV" = scalar copy psum->sbuf
# bf16 then DVE 2x pair-trick, "SG" = scalar copy then gpsimd multiply.
ROUTE1 = ("V", "SV", "SG", "SG")
ROUTE2 = ("V", "SV", "SG", "SG")


def _legalize_waits(nc):
    """walrus rejects >1 sync wait per instruction; hoist extras onto NoOps."""
    n = 0
    for fn in nc.m.functions:
        for bb in fn.blocks:
            insts = bb.instructions
            out = []
            changed = False
            for inst in insts:
                si = inst.sync_info
                if si is not None and len(si.on_wait) > 1:
                    waits = list(si.on_wait)
                    for w in waits[:-1]:
                        n += 1
                        out.append(
                            mybir.InstNoOp(
                                name=f"I-lw-{n}",
                                engine=inst.engine,
                                sync_info=bass_rust.SyncInfo(on_wait=[w], on_update=[]),
                            )
                        )
                    inst.sync_info = bass_rust.SyncInfo(
                        on_wait=[waits[-1]], on_update=list(si.on_update)
                    )
                    changed = True
                out.append(inst)
            if changed:
                bb.instructions = out
    return n


def _enable_ldw_opt():
    """Enable walrus LDW dedup (safe: multi-wait legalization keeps
    instructions single-wait)."""
    if _CACHED.get("ldw_patched"):
        return
    orig = _bu.bir_verify_and_optimise

    def patched(tmpdir, inp="bir.json", outp="file.neff", arch=None, *, dve_root=None):
        import pathlib

        cmd = [
            _bu.get_walrus_driver(),
            "--pass",
            "birverifier,runtime_memory_reservation,lower_act,lower_dve,"
            "lower_ap_offset,codegen,neff_packager",
            "-i", inp,
            "--neff-output-filename", outp,
            "--enable-birsim=true", "--mem-mode=physical", "--policy=0",
            "--enable-ldw-opt=true",
            "--assign-static-dmas-to-sp=false",
            "--dram-page-size=256",
            "--enable-neff-debug-info=true",
            "--jobs", "8",
            *_bu.get_walrus_args(
                _bu.get_bir_arch(tmpdir, inp) if arch is None else arch,
                tmpdir, dve_root=dve_root,
            ),
        ]
        r = _bu.run_command(cmd, cwd=tmpdir)
        if r is not None:
            (pathlib.Path(tmpdir) / "log.txt").write_text(r.stdout)
        return f"{tmpdir}/{outp}"

    _bu.bir_verify_and_optimise = patched
    _CACHED["ldw_patched"] = True


def _build_nc():
    nc = bass.Bass()

    f16_sh = nc.declare_dram_parameter("f16_sh", [SHARD, D], CDT, isOutput=False)
    pf16_sh = nc.declare_dram_parameter("pf16_sh", [SHARD, D], CDT, isOutput=False)
    rhs_f = nc.declare_dram_parameter("rhs_f", [128, NCHUNK * 160], CDT, isOutput=False)
    bias_f = nc.declare_dram_parameter("bias_f", [1, 160], CDT, isOutput=False)
    rhs_pf = nc.declare_dram_parameter("rhs_pf", [128, NCHUNK * LOW], CDT, isOutput=False)
    bias_pf = nc.declare_dram_parameter("bias_pf", [LOW, 1], F32, isOutput=False)
    wp = nc.declare_dram_parameter("wp", [LOW, 2 * DIM], CDT, isOutput=False)
    fin1 = nc.declare_dram_parameter("fin1", [LOW, D], CDT, isOutput=False)
    fin2 = nc.declare_dram_parameter("fin2", [MID + 1, D], CDT, isOutput=False)
    out_sh = nc.declare_dram_parameter("out_sh", [SHARD, D], CDT, isOutput=True)

    Relu = mybir.ActivationFunctionType.Relu
    Ident = mybir.ActivationFunctionType.Identity

    with tile.TileContext(nc) as tc:
        with (
            tc.tile_pool(name="wpool", bufs=1) as wpool,
            tc.tile_pool(name="main", bufs=3) as main,
            tc.tile_pool(name="prod", bufs=2) as prodp,
            tc.tile_pool(name="parsb", bufs=3) as parsb,
            tc.tile_pool(name="small", bufs=3) as small,
            tc.tile_pool(name="outp", bufs=2) as outp,
            tc.tile_pool(name="lowps", bufs=2, space="PSUM") as lowps,
            tc.tile_pool(name="pst", bufs=1, space="PSUM") as pst,
            tc.tile_pool(name="parps", bufs=2, space="PSUM") as parps,
            tc.tile_pool(name="outps", bufs=1, space="PSUM") as outps,
        ):
            # ---- one-time constants / weights ----
            ident_c = wpool.tile([128, 128], CDT)
            make_identity(nc, ident_c)
            ident_f = wpool.tile([128, 128], F32)
            make_identity(nc, ident_f)
            ones_row = wpool.tile([1, 160], CDT)
            nc.gpsimd.memset(ones_row, 1.0)

            rhs_f_sb = wpool.tile([128, NCHUNK, 160], CDT)
            rhs_pf_sb = wpool.tile([128, NCHUNK, LOW], CDT)
            bias_f_sb = wpool.tile([1, 160], CDT)
            bias_pf_sb = wpool.tile([LOW, 1], F32)
            wp_sb = wpool.tile([LOW, 2 * DIM], CDT)
            fin1_sb = wpool.tile([LOW, D], CDT)
            fin2_sb = wpool.tile([MID + 1, D], CDT)

            def load_tile(t):
                row = slice(t * TILE_B, (t + 1) * TILE_B)
                fT = main.tile([128, NCHUNK, 128], CDT, tag="fT")
                nc.sync.dma_start_transpose(fT, f16_sh[row, :])
                pfT = main.tile([128, NCHUNK, 128], CDT, tag="pfT")
                nc.scalar.dma_start(pfT, pf16_sh[row, :], transpose=True)
                return row, fT, pfT

            def low_phase(loaded):
                row, fT, pfT = loaded
                rT = main.tile([128, NCHUNK, 128], CDT, tag="rT")
                nc.gpsimd.tensor_add(rT, fT, pfT)
                # f_low (+ h_bias fold), natural layout
                ps_fl = lowps.tile([128, 160], F32, tag="low")
                for c in range(NCHUNK):
                    nc.tensor.matmul(
                        ps_fl, lhsT=fT[:, c, :], rhs=rhs_f_sb[:, c, :],
                        start=(c == 0), stop=False,
                    )
                nc.tensor.matmul(
                    ps_fl, lhsT=ones_row[:, :128], rhs=bias_f_sb,
                    start=False, stop=True,
                )
                f_low = small.tile([128, LOW], CDT, tag="flow")
                nc.scalar.copy(f_low, ps_fl[:, :LOW])
                h_bias = small.tile([128, MID], F32, tag="hbias")
                nc.scalar.copy(h_bias, ps_fl[:, LOW:])

                # pf_lowT [j, b] with bias folded into the psum->sbuf copy
                ps_pl = lowps.tile([128, 128], F32, tag="low")
                for c in range(NCHUNK):
                    nc.tensor.matmul(
                        ps_pl, lhsT=rhs_pf_sb[:, c, :], rhs=pfT[:, c, :],
                        start=(c == 0), stop=(c == NCHUNK - 1),
                    )
                pf_lowT = small.tile([128, 128], CDT, tag="pflT")
                nc.scalar.activation(pf_lowT, ps_pl, Ident, bias=bias_pf_sb)
                return row, rT, f_low, h_bias, pf_lowT

            def params_half(pf_lowT, half, dup_sb, routes, prod_tag):
                """Generate one params half (4096 cols) and multiply by the
                pair-duplicated activation; returns prod tile [128, 4096]."""
                prod = prodp.tile([128, DIM], CDT, tag=prod_tag)
                base = half * DIM
                # half 0: cols (m, l) m-outer l-inner; act = f_low bcast over m
                # half 1: cols (l, m) l-outer m-inner; act = h bcast over l
                nsub, sub = (8, 128) if half == 0 else (32, 32)
                for c in range(4):
                    ps = parps.tile([128, 1024], F32, tag="par")
                    for k in range(2):
                        sl = slice(base + c * 1024 + k * 512, base + c * 1024 + (k + 1) * 512)
                        nc.tensor.matmul(
                            ps[:, k * 512:(k + 1) * 512], lhsT=pf_lowT,
                            rhs=wp_sb[:, sl], start=True, stop=True,
                        )
                    o3 = prod[:, c * 1024:(c + 1) * 1024].rearrange(
                        "p (a b) -> p a b", a=nsub)
                    act = dup_sb.unsqueeze(1).broadcast_to([128, nsub, sub])
                    route = routes[c]
                    if route == "V":
                        nc.vector.tensor_mul(
                            o3, ps.rearrange("p (a b) -> p a b", a=nsub), act)
                    else:
                        pcp = parsb.tile([128, 1024], CDT, tag="pcp")
                        nc.scalar.copy(pcp, ps)
                        i3 = pcp.rearrange("p (a b) -> p a b", a=nsub)
                        if route == "SV":
                            nc.vector.tensor_mul(o3, i3, act)
                        else:
                            nc.gpsimd.tensor_mul(o3, i3, act)
                return prod

            def stage_a(lowstate):
                row, rT, f_low, h_bias, pf_lowT = lowstate
                prod1 = params_half(pf_lowT, 0, f_low, ROUTE1, "prod1")
                h16 = small.tile([128, MID], F32, tag="h16")
                nc.vector.tensor_reduce(
                    h16, prod1.rearrange("p (m l) -> p m l", l=LOW),
                    axis=mybir.AxisListType.X, op=mybir.AluOpType.add)
                nc.vector.tensor_add(h16, h16, h_bias)
                h_sb = small.tile([128, MID], CDT, tag="hsb")
                nc.scalar.activation(h_sb, h16, Relu)
                prod2 = params_half(pf_lowT, 1, h_sb, ROUTE2, "prod2")
                g32 = small.tile([128, LOW], F32, tag="g32")
                nc.vector.tensor_reduce(
                    g32, prod2.rearrange("p (l m) -> p l m", m=MID),
                    axis=mybir.AxisListType.X, op=mybir.AluOpType.add)
                return row, rT, h_sb, g32

            def stage_b(state):
                row, rT, h_sb, g32 = state
                # hT_ext = [h.T ; ones]
                ps_ht = pst.tile([MID, 128], CDT, tag="pst")
                nc.tensor.transpose(ps_ht, h_sb, ident_c)
                hT_ext = small.tile([MID + 1, 128], CDT, tag="hTe")
                nc.scalar.copy(hT_ext[:MID, :], ps_ht)
                nc.gpsimd.memset(hT_ext[MID:MID + 1, :], 1.0)
                # gT (f32 transpose, psum f32 -> bf16 copy)
                ps_gt = pst.tile([128, 128], F32, tag="pst")
                nc.tensor.transpose(ps_gt, g32, ident_f)
                gT_sb = small.tile([128, 128], CDT, tag="gTs")
                nc.scalar.copy(gT_sb, ps_gt)

                out16 = outp.tile([128, D], CDT, tag="o16")
                for hf in range(2):
                    sl = slice(hf * 512, (hf + 1) * 512)
                    ps_o = outps.tile([128, 512], F32, tag="out")
                    nc.tensor.matmul(
                        ps_o, lhsT=gT_sb, rhs=fin1_sb[:, sl],
                        start=True, stop=False,
                    )
                    nc.tensor.matmul(
                        ps_o, lhsT=hT_ext, rhs=fin2_sb[:, sl],
                        start=False, stop=False,
                    )
                    for k in range(4):
                        c = hf * 4 + k
                        nc.tensor.matmul(
                            ps_o[:, k * 128:(k + 1) * 128], lhsT=rT[:, c, :],
                            rhs=ident_c, start=False, stop=(k == 3),
                        )
                    nc.scalar.copy(out16[:, sl], ps_o)
                nc.sync.dma_start(out_sh[row, :], out16)

            nc.scalar.dma_start(rhs_f_sb.rearrange("p c n -> p (c n)"), rhs_f[:, :])
            nc.scalar.dma_start(rhs_pf_sb.rearrange("p c n -> p (c n)"), rhs_pf[:, :])
            nc.scalar.dma_start(bias_f_sb, bias_f[:, :])
            nc.scalar.dma_start(bias_pf_sb, bias_pf[:, :])
            loads = [load_tile(0)]
            nc.scalar.dma_start(wp_sb[:, :DIM], wp[:, :DIM])
            nc.sync.dma_start(wp_sb[:, DIM:], wp[:, DIM:])
            loads.append(load_tile(1))
            nc.scalar.dma_start(fin1_sb, fin1[:, :])
            nc.scalar.dma_start(fin2_sb, fin2[:, :])
            pending = []
            for t in range(NT):
                if t + 2 < NT:
                    loads.append(load_tile(t + 2))
                st = stage_a(low_phase(loads.pop(0)))
                pending.append(st)
                if len(pending) > 1:
                    stage_b(pending.pop(0))
            for st in pending:
                stage_b(st)

    _legalize_waits(nc)
    return nc


def _host_prep(proj_f_w, proj_f_b, proj_pf_w, proj_pf_b, proj_f2_w, proj_f2_b,
               pg_w, pg_b):
    B1 = pg_b[:DIM].reshape(LOW, MID)
    B2 = pg_b[DIM:].reshape(MID, LOW)
    c = np.ascontiguousarray
    return {
        "rhs_f": c(np.concatenate([proj_f_w.T, proj_f_w.T @ B1], axis=1)
                   .reshape(NCHUNK, 128, 160).transpose(1, 0, 2).reshape(128, NCHUNK * 160)
                   .astype(NP_CDT)),
        "bias_f": c(np.concatenate([proj_f_b, proj_f_b @ B1])[None, :].astype(NP_CDT)),
        "rhs_pf": c(proj_pf_w.T.reshape(NCHUNK, 128, LOW).transpose(1, 0, 2)
                    .reshape(128, NCHUNK * LOW).astype(NP_CDT)),
        "bias_pf": c(proj_pf_b[:, None].astype(np.float32)),
        "wp": c(np.concatenate([
            pg_w[:DIM].reshape(LOW, MID, LOW).transpose(2, 1, 0).reshape(LOW, DIM),
            pg_w[DIM:].reshape(MID, LOW, LOW).transpose(2, 1, 0).reshape(LOW, DIM),
        ], axis=1).astype(NP_CDT)),
        "fin1": c(proj_f2_w.T.astype(NP_CDT)),
        "fin2": c(np.concatenate([B2 @ proj_f2_w.T, proj_f2_b[None, :]], axis=0).astype(NP_CDT)),
    }


def kernel(f, pf, proj_f_w, proj_f_b, proj_pf_w, proj_pf_b, proj_f2_w, proj_f2_b,
           pg_w, pg_b):
    f = np.ascontiguousarray(np.asarray(f, dtype=np.float32))
    pf = np.ascontiguousarray(np.asarray(pf, dtype=np.float32))
    f16 = f.astype(NP_CDT)
    pf16 = pf.astype(NP_CDT)
    weights = _host_prep(
        np.asarray(proj_f_w, np.float32), np.asarray(proj_f_b, np.float32),
        np.asarray(proj_pf_w, np.float32), np.asarray(proj_pf_b, np.float32),
        np.asarray(proj_f2_w, np.float32), np.asarray(proj_f2_b, np.float32),
        np.asarray(pg_w, np.float32), np.asarray(pg_b, np.float32),
    )

    if "nc" not in _CACHED:
        _CACHED["nc"] = _build_nc()
    nc = _CACHED["nc"]

    in_maps = []
    for i in range(N_CORES):
        m = dict(weights)
        m["f16_sh"] = f16[i * SHARD:(i + 1) * SHARD]
        m["pf16_sh"] = pf16[i * SHARD:(i + 1) * SHARD]
        in_maps.append(m)

    res = run_bass_kernel_spmd(nc, in_maps, core_ids=list(range(N_CORES)))
    out = np.concatenate(
        [res.results[i]["out_sh"].astype(np.float32) for i in range(N_CORES)], axis=0
    )
    return out


# revision 12
# speedup vs baseline: 1.0388x; 1.0388x over previous
"""Trainium2 Bass kernel for nn_DynamicFc (per-sample dynamic MLP).

Data-parallel over 8 cores (batch 8192 -> 8 x 1024), 8 tiles of 128/core.

Per 128-sample tile (natural [batch, feat] layout):
  psum_fl = f @ [Wf.T | Wf.T@B1] + [bf | bf@B1]        (PE, 160 cols)
  pf_lowT = (Wpf.T).T-chunks @ pfT-chunks + bpf        (PE, transposed [j,b])
  p1 = pf_low @ W1 (l-major cols), p2 = pf_low @ W2 (m-major)   (PE)
  prod1 = p1 * f_low   (DVE/GPSIMD, routed per chunk; scalar copies
                        psum->sbuf bf16 so DVE can run the 2x mode via a
                        pair-duplicated broadcast operand)
  h = relu(tree_reduce_l(prod1) + h_bias)              (DVE tree + scalar)
  prod2 = p2 * h; g = tree_reduce_m(prod2)             (same scheme)
  out = gT.T @ fin1 + [hT;1].T @ fin2 + r16            (PE + DVE adds)
Residual r16 = (f+pf) in bf16 from host; output stored bf16, upcast on host.
"""

import os
import sys

import numpy as np

for _p in ("/opt/trn_rl_repo", "/root/.axon_site/_ro/trn_rl_repo"):
    if os.path.isdir(_p) and _p not in sys.path:
        sys.path.insert(0, _p)

import bass_rust
import concourse.bass as bass
import concourse.bass_utils as _bu
import concourse.mybir as mybir
import concourse.tile as tile
from concourse.bass_utils import run_bass_kernel_spmd
from concourse.masks import make_identity

import ml_dtypes

N_CORES = 8
B, D, LOW, MID = 8192, 1024, 128, 32
DIM = LOW * MID  # 4096
SHARD = B // N_CORES  # 1024
TILE_B = 128
NT = SHARD // TILE_B  # 8
NCHUNK = D // 128  # 8

F32 = mybir.dt.float32
CDT = mybir.dt.bfloat16
NP_CDT = ml_dtypes.bfloat16

_CACHED = {}

# per-chunk routing for the params first-touch multiply (4 chunks of 1024
# per half): "V" = DVE direct from psum (1x), "SV" = scalar copy psum->sbuf
# bf16 then DVE 2x pair-trick, "SG" = scalar copy then gpsimd multiply.
ROUTE1 = ("V", "SV", "SV", "SG")
ROUTE2 = ("V", "SV", "SV", "SG")


def _legalize_waits(nc):
    """walrus rejects >1 sync wait per instruction; hoist extras onto NoOps."""
    n = 0
    for fn in nc.m.functions:
        for bb in fn.blocks:
            insts = bb.instructions
            out = []
            changed = False
            for inst in insts:
                si = inst.sync_info
                if si is not None and len(si.on_wait) > 1:
                    waits = list(si.on_wait)
                    for w in waits[:-1]:
                        n += 1
                        out.append(
                            mybir.InstNoOp(
                                name=f"I-lw-{n}",
                                engine=inst.engine,
                                sync_info=bass_rust.SyncInfo(on_wait=[w], on_update=[]),
                            )
                        )
                    inst.sync_info = bass_rust.SyncInfo(
                        on_wait=[waits[-1]], on_update=list(si.on_update)
                    )
                    changed = True
                out.append(inst)
            if changed:
                bb.instructions = out
    return n


def _enable_ldw_opt():
    """Enable walrus LDW dedup (safe: multi-wait legalization keeps
    instructions single-wait)."""
    if _CACHED.get("ldw_patched"):
        return
    orig = _bu.bir_verify_and_optimise

    def patched(tmpdir, inp="bir.json", outp="file.neff", arch=None, *, dve_root=None):
        import pathlib

        cmd = [
            _bu.get_walrus_driver(),
            "--pass",
            "birverifier,runtime_memory_reservation,lower_act,lower_dve,"
            "lower_ap_offset,codegen,neff_packager",
            "-i", inp,
            "--neff-output-filename", outp,
            "--enable-birsim=true", "--mem-mode=physical", "--policy=0",
            "--enable-ldw-opt=true",
            "--assign-static-dmas-to-sp=false",
            "--dram-page-size=256",
            "--enable-neff-debug-info=true",
            "--jobs", "8",
            *_bu.get_walrus_args(
                _bu.get_bir_arch(tmpdir, inp) if arch is None else arch,
                tmpdir, dve_root=dve_root,
            ),
        ]
        r = _bu.run_command(cmd, cwd=tmpdir)
        if r is not None:
            (pathlib.Path(tmpdir) / "log.txt").write_text(r.stdout)
        return f"{tmpdir}/{outp}"

    _bu.bir_verify_and_optimise = patched
    _CACHED["ldw_patched"] = True


def _build_nc():
    nc = bass.Bass()

    f16_sh = nc.declare_dram_parameter("f16_sh", [SHARD, D], CDT, isOutput=False)
    pf16_sh = nc.declare_dram_parameter("pf16_sh", [SHARD, D], CDT, isOutput=False)
    rhs_f = nc.declare_dram_parameter("rhs_f", [128, NCHUNK * 160], CDT, isOutput=False)
    bias_f = nc.declare_dram_parameter("bias_f", [1, 160], CDT, isOutput=False)
    rhs_pf = nc.declare_dram_parameter("rhs_pf", [128, NCHUNK * LOW], CDT, isOutput=False)
    bias_pf = nc.declare_dram_parameter("bias_pf", [LOW, 1], F32, isOutput=False)
    wp = nc.declare_dram_parameter("wp", [LOW, 2 * DIM], CDT, isOutput=False)
    fin1 = nc.declare_dram_parameter("fin1", [LOW, D], CDT, isOutput=False)
    fin2 = nc.declare_dram_parameter("fin2", [MID + 1, D], CDT, isOutput=False)
    out_sh = nc.declare_dram_parameter("out_sh", [SHARD, D], CDT, isOutput=True)

    Relu = mybir.ActivationFunctionType.Relu
    Ident = mybir.ActivationFunctionType.Identity

    with tile.TileContext(nc) as tc:
        with (
            tc.tile_pool(name="wpool", bufs=1) as wpool,
            tc.tile_pool(name="main", bufs=4) as main,
            tc.tile_pool(name="prod", bufs=2) as prodp,
            tc.tile_pool(name="parsb", bufs=3) as parsb,
            tc.tile_pool(name="small", bufs=3) as small,
            tc.tile_pool(name="outp", bufs=2) as outp,
            tc.tile_pool(name="lowps", bufs=2, space="PSUM") as lowps,
            tc.tile_pool(name="parps", bufs=2, space="PSUM") as parps,
            tc.tile_pool(name="outps", bufs=2, space="PSUM") as outps,
        ):
            # ---- one-time constants / weights ----
            ident_c = wpool.tile([128, 128], CDT)
            make_identity(nc, ident_c)
            ident_f = wpool.tile([128, 128], F32)
            make_identity(nc, ident_f)
            ones_row = wpool.tile([1, 160], CDT)
            nc.gpsimd.memset(ones_row, 1.0)

            rhs_f_sb = wpool.tile([128, NCHUNK, 160], CDT)
            rhs_pf_sb = wpool.tile([128, NCHUNK, LOW], CDT)
            bias_f_sb = wpool.tile([1, 160], CDT)
            bias_pf_sb = wpool.tile([LOW, 1], F32)
            wp_sb = wpool.tile([LOW, 2 * DIM], CDT)
            fin1_sb = wpool.tile([LOW, D], CDT)
            fin2_sb = wpool.tile([MID + 1, D], CDT)

            def load_tile(t):
                row = slice(t * TILE_B, (t + 1) * TILE_B)
                fT = main.tile([128, NCHUNK, 128], CDT, tag="fT")
                nc.sync.dma_start_transpose(fT, f16_sh[row, :])
                pfT = main.tile([128, NCHUNK, 128], CDT, tag="pfT")
                nc.sync.dma_start_transpose(pfT, pf16_sh[row, :])
                return row, fT, pfT

            def low_phase(loaded):
                row, fT, pfT = loaded
                rT = main.tile([128, NCHUNK, 128], CDT, tag="rT")
                nc.gpsimd.tensor_add(rT, fT, pfT)
                # f_low (+ h_bias fold), natural layout
                ps_fl = lowps.tile([128, 160], F32, tag="low")
                for c in range(NCHUNK):
                    nc.tensor.matmul(
                        ps_fl, lhsT=fT[:, c, :], rhs=rhs_f_sb[:, c, :],
                        start=(c == 0), stop=False,
                    )
                nc.tensor.matmul(
                    ps_fl, lhsT=ones_row[:, :128], rhs=bias_f_sb,
                    start=False, stop=True,
                )
                f_low = small.tile([128, LOW], CDT, tag="flow")
                nc.scalar.copy(f_low, ps_fl[:, :LOW])
                h_bias = small.tile([128, MID], F32, tag="hbias")
                nc.scalar.copy(h_bias, ps_fl[:, LOW:])

                # pf_lowT [j, b] with bias folded into the psum->sbuf copy
                ps_pl = lowps.tile([128, 128], F32, tag="low")
                for c in range(NCHUNK):
                    nc.tensor.matmul(
                        ps_pl, lhsT=rhs_pf_sb[:, c, :], rhs=pfT[:, c, :],
                        start=(c == 0), stop=(c == NCHUNK - 1),
                    )
                pf_lowT = small.tile([128, 128], CDT, tag="pflT")
                nc.scalar.activation(pf_lowT, ps_pl, Ident, bias=bias_pf_sb)
                return row, rT, f_low, h_bias, pf_lowT

            def params_half(pf_lowT, half, dup_sb, routes, prod_tag):
                """Generate one params half (4096 cols) and multiply by the
                pair-duplicated activation; returns prod tile [128, 4096]."""
                prod = prodp.tile([128, DIM], CDT, tag=prod_tag)
                base = half * DIM
                # half 0: cols (m, l) m-outer l-inner; act = f_low bcast over m
                # half 1: cols (l, m) l-outer m-inner; act = h bcast over l
                nsub, sub = (8, 128) if half == 0 else (32, 32)
                for c in range(4):
                    ps = parps.tile([128, 1024], F32, tag="par")
                    for k in range(2):
                        sl = slice(base + c * 1024 + k * 512, base + c * 1024 + (k + 1) * 512)
                        nc.tensor.matmul(
                            ps[:, k * 512:(k + 1) * 512], lhsT=pf_lowT,
                            rhs=wp_sb[:, sl], start=True, stop=True,
                        )
                    o3 = prod[:, c * 1024:(c + 1) * 1024].rearrange(
                        "p (a b) -> p a b", a=nsub)
                    act = dup_sb.unsqueeze(1).broadcast_to([128, nsub, sub])
                    route = routes[c]
                    if route == "V":
                        nc.vector.tensor_mul(
                            o3, ps.rearrange("p (a b) -> p a b", a=nsub), act)
                    else:
                        pcp = parsb.tile([128, 1024], CDT, tag="pcp")
                        nc.scalar.copy(pcp, ps)
                        i3 = pcp.rearrange("p (a b) -> p a b", a=nsub)
                        if route == "SV":
                            nc.vector.tensor_mul(o3, i3, act)
                        else:
                            nc.gpsimd.tensor_mul(o3, i3, act)
                return prod

            def stage_a(lowstate):
                row, rT, f_low, h_bias, pf_lowT = lowstate
                prod1 = params_half(pf_lowT, 0, f_low, ROUTE1, "prod1")
                h16 = small.tile([128, MID], F32, tag="h16")
                p1v = prod1.rearrange("p (m l) -> p m l", l=LOW)
                nc.vector.tensor_reduce(
                    h16[:, :16], p1v[:, :16, :],
                    axis=mybir.AxisListType.X, op=mybir.AluOpType.add)
                nc.vector.tensor_reduce(
                    h16[:, 16:], p1v[:, 16:, :],
                    axis=mybir.AxisListType.X, op=mybir.AluOpType.add)
                nc.vector.tensor_add(h16, h16, h_bias)
                h_sb = small.tile([128, MID], CDT, tag="hsb")
                nc.scalar.activation(h_sb, h16, Relu)
                prod2 = params_half(pf_lowT, 1, h_sb, ROUTE2, "prod2")
                g32 = small.tile([128, LOW], F32, tag="g32")
                p2v = prod2.rearrange("p (l m) -> p l m", m=MID)
                nc.vector.tensor_reduce(
                    g32[:, :64], p2v[:, :64, :],
                    axis=mybir.AxisListType.X, op=mybir.AluOpType.add)
                nc.vector.tensor_reduce(
                    g32[:, 64:], p2v[:, 64:, :],
                    axis=mybir.AxisListType.X, op=mybir.AluOpType.add)
                return row, rT, h_sb, g32

            def stage_b(state):
                row, rT, h_sb, g32 = state
                # hT_ext = [h.T ; ones]
                ps_ht = lowps.tile([MID, 128], CDT, tag="low")
                nc.tensor.transpose(ps_ht, h_sb, ident_c)
                hT_ext = small.tile([MID + 1, 128], CDT, tag="hTe")
                nc.scalar.copy(hT_ext[:MID, :], ps_ht)
                nc.gpsimd.memset(hT_ext[MID:MID + 1, :], 1.0)
                # gT (f32 transpose, psum f32 -> bf16 copy)
                ps_gt = lowps.tile([128, 128], F32, tag="low")
                nc.tensor.transpose(ps_gt, g32, ident_f)
                gT_sb = small.tile([128, 128], CDT, tag="gTs")
                nc.scalar.copy(gT_sb, ps_gt)

                out16 = outp.tile([128, D], CDT, tag="o16")
                for hf in range(2):
                    sl = slice(hf * 512, (hf + 1) * 512)
                    ps_o = outps.tile([128, 512], F32, tag="out")
                    nc.tensor.matmul(
                        ps_o, lhsT=gT_sb, rhs=fin1_sb[:, sl],
                        start=True, stop=False,
                    )
                    nc.tensor.matmul(
                        ps_o, lhsT=hT_ext, rhs=fin2_sb[:, sl],
                        start=False, stop=False,
                    )
                    for k in range(4):
                        c = hf * 4 + k
                        nc.tensor.matmul(
                            ps_o[:, k * 128:(k + 1) * 128], lhsT=rT[:, c, :],
                            rhs=ident_c, start=False, stop=(k == 3),
                        )
                    nc.scalar.copy(out16[:, sl], ps_o)
                nc.sync.dma_start(out_sh[row, :], out16)

            nc.scalar.dma_start(rhs_f_sb.rearrange("p c n -> p (c n)"), rhs_f[:, :])
            nc.scalar.dma_start(rhs_pf_sb.rearrange("p c n -> p (c n)"), rhs_pf[:, :])
            nc.scalar.dma_start(bias_f_sb, bias_f[:, :])
            nc.scalar.dma_start(bias_pf_sb, bias_pf[:, :])
            loads = [load_tile(0)]
            nc.scalar.dma_start(wp_sb[:, :DIM], wp[:, :DIM])
            nc.sync.dma_start(wp_sb[:, DIM:], wp[:, DIM:])
            loads.append(load_tile(1))
            nc.scalar.dma_start(fin1_sb, fin1[:, :])
            nc.scalar.dma_start(fin2_sb, fin2[:, :])
            loads.append(load_tile(2))
            pending = []
            for t in range(NT):
                if t + 3 < NT:
                    loads.append(load_tile(t + 3))
                st = stage_a(low_phase(loads.pop(0)))
                pending.append(st)
                if len(pending) > 1:
                    stage_b(pending.pop(0))
            for st in pending:
                stage_b(st)

    _legalize_waits(nc)
    return nc


def _host_prep(proj_f_w, proj_f_b, proj_pf_w, proj_pf_b, proj_f2_w, proj_f2_b,
               pg_w, pg_b):
    B1 = pg_b[:DIM].reshape(LOW, MID)
    B2 = pg_b[DIM:].reshape(MID, LOW)
    c = np.ascontiguousarray
    return {
        "rhs_f": c(np.concatenate([proj_f_w.T, proj_f_w.T @ B1], axis=1)
                   .reshape(NCHUNK, 128, 160).transpose(1, 0, 2).reshape(128, NCHUNK * 160)
                   .astype(NP_CDT)),
        "bias_f": c(np.concatenate([proj_f_b, proj_f_b @ B1])[None, :].astype(NP_CDT)),
        "rhs_pf": c(proj_pf_w.T.reshape(NCHUNK, 128, LOW).transpose(1, 0, 2)
                    .reshape(128, NCHUNK * LOW).astype(NP_CDT)),
        "bias_pf": c(proj_pf_b[:, None].astype(np.float32)),
        "wp": c(np.concatenate([
            pg_w[:DIM].reshape(LOW, MID, LOW).transpose(2, 1, 0).reshape(LOW, DIM),
            pg_w[DIM:].reshape(MID, LOW, LOW).transpose(2, 1, 0).reshape(LOW, DIM),
        ], axis=1).astype(NP_CDT)),
        "fin1": c(proj_f2_w.T.astype(NP_CDT)),
        "fin2": c(np.concatenate([B2 @ proj_f2_w.T, proj_f2_b[None, :]], axis=0).astype(NP_CDT)),
    }


def kernel(f, pf, proj_f_w, proj_f_b, proj_pf_w, proj_pf_b, proj_f2_w, proj_f2_b,
           pg_w, pg_b):
    f = np.ascontiguousarray(np.asarray(f, dtype=np.float32))
    pf = np.ascontiguousarray(np.asarray(pf, dtype=np.float32))
    f16 = f.astype(NP_CDT)
    pf16 = pf.astype(NP_CDT)
    weights = _host_prep(
        np.asarray(proj_f_w, np.float32), np.asarray(proj_f_b, np.float32),
        np.asarray(proj_pf_w, np.float32), np.asarray(proj_pf_b, np.float32),
        np.asarray(proj_f2_w, np.float32), np.asarray(proj_f2_b, np.float32),
        np.asarray(pg_w, np.float32), np.asarray(pg_b, np.float32),
    )

    if "nc" not in _CACHED:
        _CACHED["nc"] = _build_nc()
    nc = _CACHED["nc"]

    in_maps = []
    for i in range(N_CORES):
        m = dict(weights)
        m["f16_sh"] = f16[i * SHARD:(i + 1) * SHARD]
        m["pf16_sh"] = pf16[i * SHARD:(i + 1) * SHARD]
        in_maps.append(m)

    res = run_bass_kernel_spmd(nc, in_maps, core_ids=list(range(N_CORES)))
    out = np.concatenate(
        [res.results[i]["out_sh"].astype(np.float32) for i in range(N_CORES)], axis=0
    )
    return out
